# revision 14
# baseline (speedup 1.0000x reference)
"""Multi-head attention (B=4, S=2048, D=1024, H=16) on 8 Trainium2 cores.

Sharding: data-parallel over the 4 batches x tensor-parallel over 2 groups
of 8 heads. Core c handles batch c//2, head group c%2. Each core computes
its group's slice of the out-projection; the host sums the two partial
outputs per batch.

Device-side layout (per core, matmul operands bf16):
  qhT [128, jt, S]   : q projections, head dims on partitions (2 heads/jt).
  kD  [128, h, S]    : k projections zero-PADDED to K=128 per head: head h
                       occupies partitions (h%2)*64..+64, the other 64
                       partitions are zero. Scores matmuls then contract
                       over the full 128 partitions at full streaming rate
                       (K=64 matmuls run at half rate on TRN2).
  v_sb[128, sjt, h, 66]: v in [keys, dims] layout + a ones column (64) for
                       the softmax denominators during attn@V accumulation.
  scores psum [128, 512]: keys on partitions; exp rides ScalarE out of
                       PSUM into bf16 et tiles.
  outT [1024, S]     : transposed partial out-projection, summed on host.

Schedule: per (head, query-chunk) unit the score groups are software-
pipelined: emit scores(g+1) before attn@V(g) so the in-order PE never
waits on the exp of group g.
"""
import sys

for _p in ("/opt/trn_rl_repo", "/root/.axon_site/_ro/trn_rl_repo"):
    if _p not in sys.path:
        sys.path.append(_p)

import numpy as np

import concourse.bass as bass
import concourse.tile as tile
from concourse import bacc, mybir
from concourse.bass_utils import run_bass_kernel_spmd

N_CORES = 8
B, S, DIM, H, DK = 4, 2048, 1024, 16, 64
JG = DIM // 2          # head-group width (8 heads x 64)
HPG = 8                # heads per group
F32 = mybir.dt.float32
BF16 = mybir.dt.bfloat16
FP16 = mybir.dt.float16

N_KC = DIM // 128      # contraction chunks for projections
N_JT = JG // 128       # 128-row tiles of the group width
N_SJT = S // 128       # key tiles
N_SIC = S // 512       # query chunks
VW = 66                # v row stride (64 dims + ones col + pad)

# score/exp groups: 16 sjt as [3,3,3,3,3,1]
GRP = [(0, 3), (3, 6), (6, 9), (9, 12), (12, 15), (15, 16)]


def build_program(phases="ABC"):
    """phases: subset of "ABC" (+ ablation codes) for perf decomposition:
    A=projections, B=attention, C=out-projection.
    'x' in phases: skip exp (attn@V consumes stale et tiles).
    's' in phases: skip score matmuls.
    'v' in phases: skip attn@V matmuls (den/at read stale psum).
    Correctness only holds for the default "ABC"."""
    nc = bacc.Bacc("TRN2", target_bir_lowering=False, debug=False,
                   num_devices=N_CORES)
    xqT = nc.dram_tensor("xqT", [2, 128, N_KC, 1024], BF16,
                         kind="ExternalInput").ap()
    xkT = nc.dram_tensor("xkT", [2, 128, N_KC, 1024], BF16,
                         kind="ExternalInput").ap()
    xvT = nc.dram_tensor("xvT", [2, 128, N_KC, 1024], BF16,
                         kind="ExternalInput").ap()
    wqT = nc.dram_tensor("wqT", [128, N_KC, JG], BF16,
                         kind="ExternalInput").ap()
    wkT = nc.dram_tensor("wkT", [128, N_KC, JG], BF16,
                         kind="ExternalInput").ap()
    wvT = nc.dram_tensor("wvT", [128, N_KC, JG], BF16,
                         kind="ExternalInput").ap()
    woT = nc.dram_tensor("woT", [128, N_JT, DIM], BF16,
                         kind="ExternalInput").ap()
    bq = nc.dram_tensor("bq", [128, N_JT], F32, kind="ExternalInput").ap()
    bk = nc.dram_tensor("bk", [128, N_JT], F32, kind="ExternalInput").ap()
    bvr = nc.dram_tensor("bvr", [128, JG], F32, kind="ExternalInput").ap()
    outT = nc.dram_tensor("outT", [DIM // 128, N_SIC, 128, 512], FP16,
                          kind="ExternalOutput").ap()

    with tile.TileContext(nc) as tc:
        with (
            tc.tile_pool(name="wproj", bufs=2) as wpool,
            tc.tile_pool(name="wo", bufs=1) as wopool,
            tc.tile_pool(name="xin", bufs=2) as xpool,
            tc.tile_pool(name="bias", bufs=1) as bpool,
            tc.tile_pool(name="qk", bufs=1) as qkpool,
            tc.tile_pool(name="vp", bufs=1) as vpool,
            tc.tile_pool(name="attn", bufs=2) as apool,
            tc.tile_pool(name="exp", bufs=3) as epool,
            tc.tile_pool(name="expfull", bufs=3) as efpool,
            tc.tile_pool(name="small", bufs=3) as spool,
            tc.tile_pool(name="outsb", bufs=4) as opool,
        ):
            # ---- persistent SBUF residents ----
            qhT = qkpool.tile([128, N_JT, S], BF16, tag="qhT")
            kD = qkpool.tile([128, HPG, S], BF16, tag="kD")
            v_sb = vpool.tile([128, N_SJT, HPG, VW], BF16, tag="v")
            wo_sb = wopool.tile([128, N_JT, DIM], BF16, tag="wo")
            bq_sb = bpool.tile([128, N_JT], F32, tag="bq")
            bk_sb = bpool.tile([128, N_JT], F32, tag="bk")
            bvr_sb = bpool.tile([128, JG], F32, tag="bvr")

            wk_sb = wpool.tile([128, N_KC, JG], BF16, tag="w", name="wk_sb")
            wq_sb = wpool.tile([128, N_KC, JG], BF16, tag="w", name="wq_sb")
            wv_sb = wpool.tile([128, N_KC, JG], BF16, tag="w", name="wv_sb")
            # wk split per contraction chunk: the first k matmul only needs
            # chunk 0, so it unblocks after 128KB instead of 1MB
            for _kc in range(N_KC):
                nc.scalar.dma_start(wk_sb[:, _kc, :], wkT[:, _kc, :])
            nc.scalar.dma_start(wq_sb[:], wqT[:])
            nc.scalar.dma_start(wv_sb[:], wvT[:])
            nc.sync.dma_start(bq_sb[:], bq[:])
            nc.sync.dma_start(bk_sb[:], bk[:])
            nc.sync.dma_start(bvr_sb[:], bvr[:])
            # zero the dead partition-halves of kD (even heads live on
            # partitions 0-63, odd heads on 64-127)
            for h in range(HPG):
                po = (h % 2) * 64
                nc.vector.memset(
                    kD[64 - po:128 - po, h, :].bitcast(F32), 0.0)
            # ones column for the softmax denominators (bf16 pair 1.0,1.0
            # via f32 bit pattern 0x3F803F80)
            ones_pair = float(np.frombuffer(
                np.uint32(0x3F803F80).tobytes(), np.float32)[0])
            nc.vector.memset(
                v_sb[:, :, :, 64:66].bitcast(F32), ones_pair)
            # touch Exp early so the ACT table set loads during phase A
            warm = bpool.tile([1, 2], F32, tag="warm")
            nc.vector.memset(warm[:], 0.0)
            nc.scalar.activation(warm[:], warm[:],
                                 mybir.ActivationFunctionType.Exp)

            # ---- phase A: projections (k, q) in 512-column passes, then
            # early scores+exp for 3 units interleaved with the v
            # projection (their exps ride ScalarE under the v matmuls) ----
            with tc.tile_pool(name="psA", bufs=4, space="PSUM") as psA:
                def kq_proj(w_sb, x_dram, b_sb, write_out):
                    for sh in range(2):
                        for sc in range(2):
                            xt = xpool.tile([128, N_KC, 512], BF16, tag="x")
                            nc.sync.dma_start(
                                xt[:],
                                x_dram[sh, :, :, sc * 512:(sc + 1) * 512])
                            ps4 = [psA.tile([128, 512], F32, tag="ps",
                                            name=f"ps_{j}")
                                   for j in range(4)]
                            for kc in range(N_KC):
                                for jt in range(N_JT):
                                    nc.tensor.matmul(
                                        ps4[jt][:],
                                        w_sb[:, kc, jt * 128:(jt + 1) * 128],
                                        xt[:, kc, :],
                                        start=(kc == 0), stop=(kc == N_KC - 1))
                            for jt in range(N_JT):
                                write_out(sh, sc, jt, ps4[jt], b_sb)

                def write_k(sh, sc, jt, ps, b_sb):
                    sl = slice(sh * 1024 + sc * 512, sh * 1024 + sc * 512 + 512)
                    nc.vector.tensor_scalar_add(
                        kD[0:64, 2 * jt, sl], ps[0:64, :],
                        b_sb[0:64, jt:jt + 1])
                    nc.vector.tensor_scalar_add(
                        kD[64:128, 2 * jt + 1, sl], ps[64:128, :],
                        b_sb[64:128, jt:jt + 1])

                def write_q(sh, sc, jt, ps, b_sb):
                    nc.vector.tensor_scalar_add(
                        qhT[:, jt, sh * 1024 + sc * 512:
                            sh * 1024 + sc * 512 + 512],
                        ps[:], b_sb[:, jt:jt + 1])

                if "A" in phases:
                    kq_proj(wk_sb, xkT, bk_sb, write_k)
                    kq_proj(wq_sb, xqT, bq_sb, write_q)

                SX = 3
                etfull = None

                def sx_group(u, g):
                    jt = u // 2
                    ps = psA.tile([128, 2, 512], F32, tag="sx", bufs=2)
                    for i in range(2):
                        sjt = 2 * g + i
                        nc.tensor.matmul(
                            ps[:, i, :],
                            kD[:, u, sjt * 128:(sjt + 1) * 128],
                            qhT[:, jt, 0:512],
                            start=True, stop=True)
                    if "x" not in phases:
                        nc.scalar.activation(
                            etfull[u][:, 2 * g:2 * g + 2, :], ps[:],
                            mybir.ActivationFunctionType.Exp,
                            scale=1.0 / np.sqrt(DK))

                def v_pass(sh, st8):
                    # one seq-block of the v projection: [128 seq, 512 dims]
                    xt8 = xpool.tile([128, N_KC, 128], BF16, tag="xv")
                    nc.sync.dma_start(
                        xt8[:], xvT[sh, :, :, st8 * 128:(st8 + 1) * 128])
                    pv = psA.tile([128, 512], F32, tag="ps", name="pv")
                    for kc in range(N_KC):
                        nc.tensor.matmul(
                            pv[:], xt8[:, kc, :], wv_sb[:, kc, :],
                            start=(kc == 0), stop=(kc == N_KC - 1))
                    st = sh * 8 + st8
                    nc.vector.tensor_tensor(
                        v_sb[:, st, :, 0:DK],
                        pv[:].rearrange("p (h d) -> p h d", h=HPG),
                        bvr_sb[:, :].rearrange("p (h d) -> p h d", h=HPG),
                        mybir.AluOpType.add)

                if "A" in phases and "B" in phases and "s" not in phases:
                    etfull = [efpool.tile([128, N_SJT, 512], BF16,
                                          tag="ef", name=f"ef{u}")
                              for u in range(SX)]
                    # interleave 16 v passes with 24 early score groups
                    sxg = [(u, g) for u in range(SX) for g in range(8)]
                    vps = [(sh, st8) for sh in range(2) for st8 in range(8)]
                    si_, vi = 0, 0
                    while si_ < len(sxg) or vi < len(vps):
                        if vi < len(vps):
                            v_pass(*vps[vi]); vi += 1
                        for _ in range(2):
                            if si_ < len(sxg) and si_ * 2 <= vi * 3:
                                sx_group(*sxg[si_]); si_ += 1
                elif "A" in phases:
                    for sh in range(2):
                        for st8 in range(8):
                            v_pass(sh, st8)

            # wo is first needed in phase C — load after projection weights
            nc.scalar.dma_start(wo_sb[:], woT[:])

            # ---- phases B/C: attention + out-projection, per query chunk ----
            with (
                tc.tile_pool(name="psS", bufs=2, space="PSUM") as psS,
                tc.tile_pool(name="psAO", bufs=2, space="PSUM") as psAO,
            ):
                def outproj(at_tile, sic_idx, ct):
                    po_c = psAO.tile([128, 512], F32, tag="pao", name="po_c")
                    for jc in range(N_JT):
                        nc.tensor.matmul(
                            po_c[:],
                            wo_sb[:, jc, ct * 128:(ct + 1) * 128],
                            at_tile[:, jc, :],
                            start=(jc == 0), stop=(jc == N_JT - 1))
                    ob = opool.tile([128, 512], FP16, tag="ob", name="ob")
                    nc.vector.tensor_copy(ob[:], po_c[:])
                    q = nc.sync if (sic_idx * 8 + ct) % 2 else nc.scalar
                    q.dma_start(outT[ct, sic_idx], ob[:])

                at_prev = None
                for sic in range(N_SIC if "B" in phases else 0):
                    si = slice(sic * 512, (sic + 1) * 512)
                    at_sb = apool.tile([128, N_JT, 512], BF16, tag="at")

                    def unit(h, deferred_et=None, sic=None, si=None,
                             at_sb=None):
                        jt, po = h // 2, (h % 2) * 64
                        pa = psAO.tile([128, 512], F32, tag="pao",
                                       name="pa")[:DK + 1, :]

                        def scores(g):
                            g0, g1 = GRP[g]
                            ps = psS.tile([128, 3, 512], F32, tag="ps")
                            if "s" in phases:
                                return ps
                            for i in range(g1 - g0):
                                sjt = g0 + i
                                nc.tensor.matmul(
                                    ps[:, i, :],
                                    kD[:, h, sjt * 128:(sjt + 1) * 128],
                                    qhT[:, jt, si],
                                    start=True, stop=True)
                            return ps

                        def av_one(sjt, et_slice):
                            nc.tensor.matmul(
                                pa[:], v_sb[:, sjt, h, 0:DK + 1],
                                et_slice,
                                start=(sjt == 0), stop=(sjt == N_SJT - 1))

                        if deferred_et is not None:
                            # attn@V over the pre-computed exp tile
                            if "v" not in phases:
                                for sjt in range(N_SJT):
                                    av_one(sjt, deferred_et[:, sjt, :])
                        else:
                            # software pipeline: PE order
                            # s0 s1 a0 s2 a1 s3 a2 s4 a3 s5 a4 a5
                            ps_cur = scores(0)
                            for g in range(len(GRP)):
                                g0, g1 = GRP[g]
                                et = epool.tile([128, 3, 512], BF16, tag="e")
                                if "x" not in phases:
                                    nc.scalar.activation(
                                        et[:, :g1 - g0, :],
                                        ps_cur[:, :g1 - g0, :],
                                        mybir.ActivationFunctionType.Exp,
                                        scale=1.0 / np.sqrt(DK))
                                if g + 1 < len(GRP):
                                    ps_cur = scores(g + 1)
                                if "v" not in phases:
                                    for i in range(g1 - g0):
                                        av_one(g0 + i, et[:, i, :])

                        den = spool.tile([1, 512], F32, tag="den")
                        nc.vector.reciprocal(den[:], pa[DK:DK + 1, :])
                        bc = spool.tile([DK, 512], F32, tag="bc")
                        nc.gpsimd.partition_broadcast(bc[:], den[:])
                        nc.vector.tensor_tensor(
                            at_sb[po:po + 64, jt, :], pa[:DK, :], bc[:],
                            mybir.AluOpType.mult)
                        # previous chunk's out-projection fills PE gaps
                        if "C" in phases and at_prev is not None:
                            outproj(at_prev, sic - 1, h)

                    if sic == 0 and etfull is not None:
                        order = [(3, None), (0, etfull[0]), (4, None),
                                 (1, etfull[1]), (5, None), (2, etfull[2]),
                                 (6, None), (7, None)]
                    else:
                        order = [(h, None) for h in range(HPG)]
                    for h, det in order:
                        unit(h, det, sic=sic, si=si, at_sb=at_sb)
                    at_prev = at_sb
                if "C" in phases and at_prev is not None:
                    for ct in range(DIM // 128):
                        outproj(at_prev, N_SIC - 1, ct)
                if at_prev is None:
                    fb = opool.tile([128, 512], FP16, tag="ob", name="fb")
                    nc.vector.memset(fb[:].bitcast(F32), 0.0)
                    nc.sync.dma_start(outT[0, 0], fb[:])
    nc.compile()
    return nc


_CACHED_NC = None


def _get_program():
    global _CACHED_NC
    if _CACHED_NC is None:
        _CACHED_NC = build_program()
    return _CACHED_NC


def _make_in_maps(q, k, v, Wq, bq, Wk, bk, Wv, bv, Wo, bo):
    import ml_dtypes
    bf16 = ml_dtypes.bfloat16
    f32 = np.float32

    def chunk_x(x):
        # [S, DIM] -> transposed, pre-chunked [2, 128, N_KC, 1024] bf16
        xT = np.asarray(x, f32).T.astype(bf16)   # [DIM, S]
        return np.ascontiguousarray(
            xT.reshape(N_KC, 128, 2, 1024).transpose(2, 1, 0, 3))

    in_maps = []
    # per-batch transposed activations (shared between the 2 TP cores)
    xT = {}
    for b in range(B):
        xT[b] = (chunk_x(q[b]), chunk_x(k[b]), chunk_x(v[b]))
    wg = {}
    for g in range(2):
        js = slice(g * JG, (g + 1) * JG)

        def tile_w(W):
            # W[js, :].T = [DIM, JG] -> [128, N_KC, JG]
            wT = np.asarray(W, f32)[js, :].T.astype(bf16)
            return np.ascontiguousarray(
                wT.reshape(N_KC, 128, JG).transpose(1, 0, 2))

        woT_g = np.asarray(Wo, f32)[:, js].T.astype(bf16)   # [JG, DIM]
        wg[g] = {
            "wqT": tile_w(Wq),
            "wkT": tile_w(Wk),
            "wvT": tile_w(Wv),
            "woT": np.ascontiguousarray(
                woT_g.reshape(N_JT, 128, DIM).transpose(1, 0, 2)),
            "bq": np.ascontiguousarray(
                np.asarray(bq, f32)[js].reshape(N_JT, 128).T),
            "bk": np.ascontiguousarray(
                np.asarray(bk, f32)[js].reshape(N_JT, 128).T),
            "bvr": np.ascontiguousarray(
                np.broadcast_to(np.asarray(bv, f32)[js], (128, JG))),
        }
    for c in range(N_CORES):
        b, g = c // 2, c % 2
        m = {"xqT": xT[b][0], "xkT": xT[b][1], "xvT": xT[b][2]}
        m.update(wg[g])
        in_maps.append(m)
    return in_maps


def _gather(results, bo):
    out = np.empty((B, S, DIM), np.float32)
    bo32 = np.asarray(bo, np.float32)
    for b in range(B):
        acc = (results[2 * b]["outT"].astype(np.float32)
               + results[2 * b + 1]["outT"].astype(np.float32))
        # [ct, sic, p, s'] -> [DIM, S]
        full = acc.transpose(0, 2, 1, 3).reshape(DIM, S)
        out[b] = full.T + bo32
    return out


def kernel(q, k, v, Wq, bq, Wk, bk, Wv, bv, Wo, bo):
    import time as _time
    nc = _get_program()
    in_maps = _make_in_maps(q, k, v, Wq, bq, Wk, bk, Wv, bv, Wo, bo)
    last_err = None
    for attempt in range(3):
        try:
            res = run_bass_kernel_spmd(nc, in_maps,
                                       core_ids=list(range(N_CORES)))
            return _gather(res.results, bo)
        except Exception as e:  # transient device/tunnel errors
            last_err = e
            _time.sleep(20 * (attempt + 1))
    raise last_err


# revision 15
# speedup vs baseline: 1.3040x; 1.3040x over previous
"""Multi-head attention (B=4, S=2048, D=1024, H=16) on 8 Trainium2 cores.

Sharding: data-parallel over the 4 batches x tensor-parallel over 2 groups
of 8 heads. Core c handles batch c//2, head group c%2. Each core computes
its group's slice of the out-projection; the host sums the two partial
outputs per batch.

Device-side layout (per core, matmul operands bf16):
  qhT [128, jt, S]   : q projections, head dims on partitions (2 heads/jt).
  kD  [128, h, S]    : k projections zero-PADDED to K=128 per head: head h
                       occupies partitions (h%2)*64..+64, the other 64
                       partitions are zero. Scores matmuls then contract
                       over the full 128 partitions at full streaming rate
                       (K=64 matmuls run at half rate on TRN2).
  v_sb[128, sjt, h, 66]: v in [keys, dims] layout + a ones column (64) for
                       the softmax denominators during attn@V accumulation.
  scores psum [128, 512]: keys on partitions; exp rides ScalarE out of
                       PSUM into bf16 et tiles.
  outT [1024, S]     : transposed partial out-projection, fp16, summed on
                       host (DRAM writes are ~6x slower than reads here,
                       so halving output bytes matters).

Schedule:
  - k/q projections in 512-column passes (1 PSUM bank per output tile).
  - Early scores+exp for 3 (head, chunk-0) units are interleaved with the
    v-projection passes so ScalarE works during the otherwise exp-free
    projection phase; their attn@V is deferred into the main loop where
    it serves as PE filler between ACT-bound units.
  - Per (head, query-chunk) unit the score groups are software-pipelined:
    scores(g+1) is emitted before attn@V(g) so the in-order PE never
    waits on the exp of group g.
"""
import sys

for _p in ("/opt/trn_rl_repo", "/root/.axon_site/_ro/trn_rl_repo"):
    if _p not in sys.path:
        sys.path.append(_p)

import numpy as np

import concourse.bass as bass
import concourse.tile as tile
from concourse import bacc, mybir
from concourse.bass_utils import run_bass_kernel_spmd

N_CORES = 8
B, S, DIM, H, DK = 4, 2048, 1024, 16, 64
JG = DIM // 2          # head-group width (8 heads x 64)
HPG = 8                # heads per group
F32 = mybir.dt.float32
BF16 = mybir.dt.bfloat16
FP16 = mybir.dt.float16

N_KC = DIM // 128      # contraction chunks for projections
N_JT = JG // 128       # 128-row tiles of the group width
N_SJT = S // 128       # key tiles
N_SIC = S // 512       # query chunks
VW = 66                # v row stride (64 dims + ones col + pad)

# score/exp groups: 16 sjt as [3,3,3,3,3,1]
GRP = [(0, 3), (3, 6), (6, 9), (9, 12), (12, 15), (15, 16)]


def build_program(phases="ABC"):
    """phases: subset of "ABC" (+ ablation codes) for perf decomposition:
    A=projections, B=attention, C=out-projection.
    'x' in phases: skip exp (attn@V consumes stale et tiles).
    's' in phases: skip score matmuls.
    'v' in phases: skip attn@V matmuls (den/at read stale psum).
    Correctness only holds for the default "ABC"."""
    nc = bacc.Bacc("TRN2", target_bir_lowering=False, debug=False,
                   num_devices=N_CORES)
    xqT = nc.dram_tensor("xqT", [2, 128, N_KC, 1024], BF16,
                         kind="ExternalInput").ap()
    xkT = nc.dram_tensor("xkT", [2, 128, N_KC, 1024], BF16,
                         kind="ExternalInput").ap()
    xvT = nc.dram_tensor("xvT", [2, 128, N_KC, 1024], BF16,
                         kind="ExternalInput").ap()
    wqT = nc.dram_tensor("wqT", [128, N_KC, JG], BF16,
                         kind="ExternalInput").ap()
    wkT = nc.dram_tensor("wkT", [128, N_KC, JG], BF16,
                         kind="ExternalInput").ap()
    wvT = nc.dram_tensor("wvT", [128, N_KC, JG], BF16,
                         kind="ExternalInput").ap()
    woT = nc.dram_tensor("woT", [128, N_JT, DIM], BF16,
                         kind="ExternalInput").ap()
    bq = nc.dram_tensor("bq", [128, N_JT], F32, kind="ExternalInput").ap()
    bk = nc.dram_tensor("bk", [128, N_JT], F32, kind="ExternalInput").ap()
    bvr = nc.dram_tensor("bvr", [128, JG], F32, kind="ExternalInput").ap()
    outT = nc.dram_tensor("outT", [DIM // 128, N_SIC, 128, 512], FP16,
                          kind="ExternalOutput").ap()

    with tile.TileContext(nc) as tc:
        with (
            tc.tile_pool(name="wproj", bufs=2) as wpool,
            tc.tile_pool(name="wo", bufs=1) as wopool,
            tc.tile_pool(name="xin", bufs=2) as xpool,
            tc.tile_pool(name="bias", bufs=1) as bpool,
            tc.tile_pool(name="qk", bufs=1) as qkpool,
            tc.tile_pool(name="vp", bufs=1) as vpool,
            tc.tile_pool(name="attn", bufs=2) as apool,
            tc.tile_pool(name="exp", bufs=3) as epool,
            tc.tile_pool(name="expfull", bufs=3) as efpool,
            tc.tile_pool(name="small", bufs=3) as spool,
            tc.tile_pool(name="outsb", bufs=4) as opool,
        ):
            # ---- persistent SBUF residents ----
            qhT = qkpool.tile([128, N_JT, S], BF16, tag="qhT")
            kD = qkpool.tile([128, HPG, S], BF16, tag="kD")
            v_sb = vpool.tile([128, N_SJT, HPG, VW], BF16, tag="v")
            wo_sb = wopool.tile([128, N_JT, DIM], BF16, tag="wo")
            bq_sb = bpool.tile([128, N_JT], F32, tag="bq")
            bk_sb = bpool.tile([128, N_JT], F32, tag="bk")
            bvr_sb = bpool.tile([128, JG], F32, tag="bvr")

            wk_sb = wpool.tile([128, N_KC, JG], BF16, tag="w", name="wk_sb")
            wq_sb = wpool.tile([128, N_KC, JG], BF16, tag="w", name="wq_sb")
            wv_sb = wpool.tile([128, N_KC, JG], BF16, tag="w", name="wv_sb")
            # wk split per contraction chunk: the first k matmul only needs
            # chunk 0, so it unblocks after 128KB instead of 1MB
            for _kc in range(N_KC):
                nc.scalar.dma_start(wk_sb[:, _kc, :], wkT[:, _kc, :])
            nc.scalar.dma_start(wq_sb[:], wqT[:])
            nc.scalar.dma_start(wv_sb[:], wvT[:])
            nc.sync.dma_start(bq_sb[:], bq[:])
            nc.sync.dma_start(bk_sb[:], bk[:])
            nc.sync.dma_start(bvr_sb[:], bvr[:])
            # zero the dead partition-halves of kD (even heads live on
            # partitions 0-63, odd heads on 64-127)
            for h in range(HPG):
                po = (h % 2) * 64
                nc.vector.memset(
                    kD[64 - po:128 - po, h, :].bitcast(F32), 0.0)
            # ones column for the softmax denominators (bf16 pair 1.0,1.0
            # via f32 bit pattern 0x3F803F80)
            ones_pair = float(np.frombuffer(
                np.uint32(0x3F803F80).tobytes(), np.float32)[0])
            nc.vector.memset(
                v_sb[:, :, :, 64:66].bitcast(F32), ones_pair)
            # touch Exp early so the ACT table set loads during phase A
            warm = bpool.tile([1, 2], F32, tag="warm")
            nc.vector.memset(warm[:], 0.0)
            nc.scalar.activation(warm[:], warm[:],
                                 mybir.ActivationFunctionType.Exp)

            # ---- phase A: projections (k, q) in 512-column passes, then
            # early scores+exp for 3 units interleaved with the v
            # projection (their exps ride ScalarE under the v matmuls) ----
            with tc.tile_pool(name="psA", bufs=4, space="PSUM") as psA:
                def kq_proj(w_sb, x_dram, b_sb, write_out):
                    for sh in range(2):
                        for sc in range(2):
                            xt = xpool.tile([128, N_KC, 512], BF16, tag="x")
                            nc.sync.dma_start(
                                xt[:],
                                x_dram[sh, :, :, sc * 512:(sc + 1) * 512])
                            ps4 = [psA.tile([128, 512], F32, tag="ps",
                                            name=f"ps_{j}")
                                   for j in range(4)]
                            for kc in range(N_KC):
                                for jt in range(N_JT):
                                    nc.tensor.matmul(
                                        ps4[jt][:],
                                        w_sb[:, kc, jt * 128:(jt + 1) * 128],
                                        xt[:, kc, :],
                                        start=(kc == 0), stop=(kc == N_KC - 1))
                            for jt in range(N_JT):
                                write_out(sh, sc, jt, ps4[jt], b_sb)

                def write_k(sh, sc, jt, ps, b_sb):
                    sl = slice(sh * 1024 + sc * 512, sh * 1024 + sc * 512 + 512)
                    nc.vector.tensor_scalar_add(
                        kD[0:64, 2 * jt, sl], ps[0:64, :],
                        b_sb[0:64, jt:jt + 1])
                    nc.vector.tensor_scalar_add(
                        kD[64:128, 2 * jt + 1, sl], ps[64:128, :],
                        b_sb[64:128, jt:jt + 1])

                def write_q(sh, sc, jt, ps, b_sb):
                    nc.vector.tensor_scalar_add(
                        qhT[:, jt, sh * 1024 + sc * 512:
                            sh * 1024 + sc * 512 + 512],
                        ps[:], b_sb[:, jt:jt + 1])

                if "A" in phases:
                    kq_proj(wk_sb, xkT, bk_sb, write_k)
                    kq_proj(wq_sb, xqT, bq_sb, write_q)

                SX = 3
                etfull = None

                def sx_group(u, g):
                    jt = u // 2
                    ps = psA.tile([128, 2, 512], F32, tag="sx", bufs=2)
                    for i in range(2):
                        sjt = 2 * g + i
                        nc.tensor.matmul(
                            ps[:, i, :],
                            kD[:, u, sjt * 128:(sjt + 1) * 128],
                            qhT[:, jt, 0:512],
                            start=True, stop=True)
                    if "x" not in phases:
                        nc.scalar.activation(
                            etfull[u][:, 2 * g:2 * g + 2, :], ps[:],
                            mybir.ActivationFunctionType.Exp,
                            scale=1.0 / np.sqrt(DK))

                def v_pass(sh, st8):
                    # one seq-block of the v projection: [128 seq, 512 dims]
                    xt8 = xpool.tile([128, N_KC, 128], BF16, tag="xv")
                    nc.sync.dma_start(
                        xt8[:], xvT[sh, :, :, st8 * 128:(st8 + 1) * 128])
                    pv = psA.tile([128, 512], F32, tag="ps", name="pv")
                    for kc in range(N_KC):
                        nc.tensor.matmul(
                            pv[:], xt8[:, kc, :], wv_sb[:, kc, :],
                            start=(kc == 0), stop=(kc == N_KC - 1))
                    st = sh * 8 + st8
                    nc.vector.tensor_tensor(
                        v_sb[:, st, :, 0:DK],
                        pv[:].rearrange("p (h d) -> p h d", h=HPG),
                        bvr_sb[:, :].rearrange("p (h d) -> p h d", h=HPG),
                        mybir.AluOpType.add)

                if "A" in phases and "B" in phases and "s" not in phases:
                    etfull = [efpool.tile([128, N_SJT, 512], BF16,
                                          tag="ef", name=f"ef{u}")
                              for u in range(SX)]
                    # interleave 16 v passes with 24 early score groups
                    sxg = [(u, g) for u in range(SX) for g in range(8)]
                    vps = [(sh, st8) for sh in range(2) for st8 in range(8)]
                    si_, vi = 0, 0
                    while si_ < len(sxg) or vi < len(vps):
                        if vi < len(vps):
                            v_pass(*vps[vi]); vi += 1
                        for _ in range(2):
                            if si_ < len(sxg) and si_ * 2 <= vi * 3:
                                sx_group(*sxg[si_]); si_ += 1
                elif "A" in phases:
                    for sh in range(2):
                        for st8 in range(8):
                            v_pass(sh, st8)

            # wo is first needed in phase C — load after projection weights
            nc.scalar.dma_start(wo_sb[:], woT[:])

            # ---- phases B/C: attention + out-projection, per query chunk ----
            with (
                tc.tile_pool(name="psS", bufs=2, space="PSUM") as psS,
                tc.tile_pool(name="psAO", bufs=2, space="PSUM") as psAO,
            ):
                def outproj(at_tile, sic_idx, ct):
                    po_c = psAO.tile([128, 512], F32, tag="pao", name="po_c")
                    for jc in range(N_JT):
                        nc.tensor.matmul(
                            po_c[:],
                            wo_sb[:, jc, ct * 128:(ct + 1) * 128],
                            at_tile[:, jc, :],
                            start=(jc == 0), stop=(jc == N_JT - 1))
                    ob = opool.tile([128, 512], FP16, tag="ob", name="ob")
                    nc.vector.tensor_copy(ob[:], po_c[:])
                    q = nc.sync if (sic_idx * 8 + ct) % 2 else nc.scalar
                    q.dma_start(outT[ct, sic_idx], ob[:])

                at_prev = None
                for sic in range(N_SIC if "B" in phases else 0):
                    si = slice(sic * 512, (sic + 1) * 512)
                    at_sb = apool.tile([128, N_JT, 512], BF16, tag="at")

                    def unit(h, deferred_et=None, sic=None, si=None,
                             at_sb=None):
                        jt, po = h // 2, (h % 2) * 64
                        pa = psAO.tile([128, 512], F32, tag="pao",
                                       name="pa")[:DK + 1, :]

                        def scores(g):
                            g0, g1 = GRP[g]
                            ps = psS.tile([128, 3, 512], F32, tag="ps")
                            if "s" in phases:
                                return ps
                            for i in range(g1 - g0):
                                sjt = g0 + i
                                nc.tensor.matmul(
                                    ps[:, i, :],
                                    kD[:, h, sjt * 128:(sjt + 1) * 128],
                                    qhT[:, jt, si],
                                    start=True, stop=True)
                            return ps

                        def av_one(sjt, et_slice):
                            nc.tensor.matmul(
                                pa[:], v_sb[:, sjt, h, 0:DK + 1],
                                et_slice,
                                start=(sjt == 0), stop=(sjt == N_SJT - 1))

                        if deferred_et is not None:
                            # attn@V over the pre-computed exp tile
                            if "v" not in phases:
                                for sjt in range(N_SJT):
                                    av_one(sjt, deferred_et[:, sjt, :])
                        else:
                            # software pipeline: PE order
                            # s0 s1 a0 s2 a1 s3 a2 s4 a3 s5 a4 a5
                            ps_cur = scores(0)
                            for g in range(len(GRP)):
                                g0, g1 = GRP[g]
                                et = epool.tile([128, 3, 512], BF16, tag="e")
                                if "x" not in phases:
                                    nc.scalar.activation(
                                        et[:, :g1 - g0, :],
                                        ps_cur[:, :g1 - g0, :],
                                        mybir.ActivationFunctionType.Exp,
                                        scale=1.0 / np.sqrt(DK))
                                if g + 1 < len(GRP):
                                    ps_cur = scores(g + 1)
                                if "v" not in phases:
                                    for i in range(g1 - g0):
                                        av_one(g0 + i, et[:, i, :])

                        den = spool.tile([1, 512], F32, tag="den")
                        nc.vector.reciprocal(den[:], pa[DK:DK + 1, :])
                        bc = spool.tile([DK, 512], F32, tag="bc")
                        nc.gpsimd.partition_broadcast(bc[:], den[:])
                        nc.vector.tensor_tensor(
                            at_sb[po:po + 64, jt, :], pa[:DK, :], bc[:],
                            mybir.AluOpType.mult)
                        # previous chunk's out-projection fills PE gaps
                        if "C" in phases and at_prev is not None:
                            outproj(at_prev, sic - 1, h)

                    if sic == 0 and etfull is not None:
                        order = [(3, None), (0, etfull[0]), (4, None),
                                 (1, etfull[1]), (5, None), (2, etfull[2]),
                                 (6, None), (7, None)]
                    else:
                        order = [(h, None) for h in range(HPG)]
                    for h, det in order:
                        unit(h, det, sic=sic, si=si, at_sb=at_sb)
                    at_prev = at_sb
                if "C" in phases and at_prev is not None:
                    for ct in range(DIM // 128):
                        outproj(at_prev, N_SIC - 1, ct)
                if at_prev is None:
                    fb = opool.tile([128, 512], FP16, tag="ob", name="fb")
                    nc.vector.memset(fb[:].bitcast(F32), 0.0)
                    nc.sync.dma_start(outT[0, 0], fb[:])
    nc.compile()
    return nc


_CACHED_NC = None


def _get_program():
    global _CACHED_NC
    if _CACHED_NC is None:
        _CACHED_NC = build_program()
    return _CACHED_NC


def _make_in_maps(q, k, v, Wq, bq, Wk, bk, Wv, bv, Wo, bo):
    import ml_dtypes
    bf16 = ml_dtypes.bfloat16
    f32 = np.float32

    def chunk_x(x):
        # [S, DIM] -> transposed, pre-chunked [2, 128, N_KC, 1024] bf16
        xT = np.asarray(x, f32).T.astype(bf16)   # [DIM, S]
        return np.ascontiguousarray(
            xT.reshape(N_KC, 128, 2, 1024).transpose(2, 1, 0, 3))

    in_maps = []
    # per-batch transposed activations (shared between the 2 TP cores)
    xT = {}
    for b in range(B):
        xT[b] = (chunk_x(q[b]), chunk_x(k[b]), chunk_x(v[b]))
    wg = {}
    for g in range(2):
        js = slice(g * JG, (g + 1) * JG)

        def tile_w(W):
            # W[js, :].T = [DIM, JG] -> [128, N_KC, JG]
            wT = np.asarray(W, f32)[js, :].T.astype(bf16)
            return np.ascontiguousarray(
                wT.reshape(N_KC, 128, JG).transpose(1, 0, 2))

        woT_g = np.asarray(Wo, f32)[:, js].T.astype(bf16)   # [JG, DIM]
        wg[g] = {
            "wqT": tile_w(Wq),
            "wkT": tile_w(Wk),
            "wvT": tile_w(Wv),
            "woT": np.ascontiguousarray(
                woT_g.reshape(N_JT, 128, DIM).transpose(1, 0, 2)),
            "bq": np.ascontiguousarray(
                np.asarray(bq, f32)[js].reshape(N_JT, 128).T),
            "bk": np.ascontiguousarray(
                np.asarray(bk, f32)[js].reshape(N_JT, 128).T),
            "bvr": np.ascontiguousarray(
                np.broadcast_to(np.asarray(bv, f32)[js], (128, JG))),
        }
    for c in range(N_CORES):
        b, g = c // 2, c % 2
        m = {"xqT": xT[b][0], "xkT": xT[b][1], "xvT": xT[b][2]}
        m.update(wg[g])
        in_maps.append(m)
    return in_maps


def _gather(results, bo):
    out = np.empty((B, S, DIM), np.float32)
    bo32 = np.asarray(bo, np.float32)
    for b in range(B):
        acc = (results[2 * b]["outT"].astype(np.float32)
               + results[2 * b + 1]["outT"].astype(np.float32))
        # [ct, sic, p, s'] -> [DIM, S]
        full = acc.transpose(0, 2, 1, 3).reshape(DIM, S)
        out[b] = full.T + bo32
    return out


def kernel(q, k, v, Wq, bq, Wk, bk, Wv, bv, Wo, bo):
    import time as _time
    nc = _get_program()
    in_maps = _make_in_maps(q, k, v, Wq, bq, Wk, bk, Wv, bv, Wo, bo)
    last_err = None
    for attempt in range(3):
        try:
            res = run_bass_kernel_spmd(nc, in_maps,
                                       core_ids=list(range(N_CORES)))
            return _gather(res.results, bo)
        except Exception as e:  # transient device/tunnel errors
            last_err = e
            _time.sleep(20 * (attempt + 1))
    raise last_err
